# revision 8
# baseline (speedup 1.0000x reference)
"""Non-local block (self-attention over 64x64 spatial map) on 8 NeuronCores.

Sharding: data-parallel over batch (B=8 -> 1 image per core). Each core runs
the full N=4096 attention for its image; no collectives.

Per-core layout strategy:
  - theta_x/phi_x stored [O=96, N=4096] fp16; scores computed TRANSPOSED
    (S^T[m, q] chunks) so softmax denominators can be produced by the PE
    itself via an appended ones-column on the PV matmul rhs.
  - exp(S - 45) on ScalarE (constant shift cancels exactly in softmax).
  - PV: lhsT = expS^T slice [128m, 128q], rhs = [g^T | ones] [128m, 97]
    -> y_u [128q, 97] accumulated over 32 m-chunks; col 96 = row sums.
  - normalize with per-partition reciprocal, transpose y back on PE,
    output projection, fp32 residual add, DMA out.
"""

import numpy as np
import ml_dtypes

B, C, O = 8, 192, 96
HH, WW = 64, 64
N = HH * WW           # 4096
NQ = 8                # q-supers of 512
QS = 512
NMC = N // 128        # 32 m-chunks
N_CORES = 8

_CACHE = {}


def _build():
    from contextlib import ExitStack
    import concourse.tile as tile
    from concourse import bacc, mybir
    from concourse.masks import make_identity

    dt = mybir.dt
    AF = mybir.ActivationFunctionType

    nc = bacc.Bacc("TRN2", target_bir_lowering=False, debug=False,
                   num_devices=N_CORES)

    x_d = nc.dram_tensor("x", [C, N], dt.float32, kind="ExternalInput").ap()
    wt_d = {}
    b_d = {}
    for p in ("theta", "phi", "g"):
        wt_d[p] = nc.dram_tensor(f"wt_{p}", [C, O], dt.float16,
                                 kind="ExternalInput").ap()
        b_d[p] = nc.dram_tensor(f"b_{p}", [O, 1], dt.float32,
                                kind="ExternalInput").ap()
    wWT_d = nc.dram_tensor("w_WT", [O, C], dt.float16, kind="ExternalInput").ap()
    bW_d = nc.dram_tensor("b_W", [C, 1], dt.float32, kind="ExternalInput").ap()
    out_d = nc.dram_tensor("out", [C, N], dt.float32, kind="ExternalOutput").ap()

    with tile.TileContext(nc) as tc:
        with ExitStack() as ctx:
            # ---------------- persistent SBUF pools ----------------
            consts = ctx.enter_context(tc.tile_pool(name="consts", bufs=1))
            xpool = ctx.enter_context(tc.tile_pool(name="x", bufs=1))
            acts = ctx.enter_context(tc.tile_pool(name="acts", bufs=1))
            expp = ctx.enter_context(tc.tile_pool(name="exp", bufs=1))
            ypool = ctx.enter_context(tc.tile_pool(name="y", bufs=8))
            ytp = ctx.enter_context(tc.tile_pool(name="yt", bufs=2))
            outp = ctx.enter_context(tc.tile_pool(name="outsb", bufs=3))

            idn16 = consts.tile([128, 128], dt.float16, tag="idn16")
            make_identity(nc, idn16[:])
            idnbf = consts.tile([128, 128], dt.bfloat16, tag="idnbf")
            make_identity(nc, idnbf[:])

            wt = {}
            bias = {}
            for p in ("theta", "phi", "g"):
                wt[p] = consts.tile([96, 2 * O], dt.float16, tag=f"wt_{p}", name=f"wt_{p}")
                nc.sync.dma_start(wt[p][:, 0:O], wt_d[p][0:96, :])
                nc.sync.dma_start(wt[p][:, O:2 * O], wt_d[p][96:192, :])
                bias[p] = consts.tile([O, 1], dt.float32, tag=f"b_{p}", name=f"b_{p}")
                nc.sync.dma_start(bias[p][:], b_d[p][:])
            wWT = consts.tile([O, C], dt.float16, tag="wWT")
            nc.sync.dma_start(wWT[:], wWT_d[:])
            bW = [consts.tile([96, 1], dt.float32, tag=f"bW{h}", name=f"bW{h}")
                  for h in (0, 1)]
            for h in (0, 1):
                nc.sync.dma_start(bW[h][:], bW_d[96 * h:96 * h + 96, :])

            # x: two row-halves [96, N] fp32 + fp16 copies for matmul
            xf = [xpool.tile([96, N], dt.float32, tag=f"xf{h}", name=f"xf{h}") for h in (0, 1)]
            xh = [xpool.tile([96, N], dt.float16, tag=f"xh{h}", name=f"xh{h}") for h in (0, 1)]
            for h in (0, 1):
                for j in range(NQ):
                    cs = slice(j * QS, (j + 1) * QS)
                    nc.sync.dma_start(xf[h][:, cs], x_d[96 * h:96 * h + 96, cs])
                    nc.vector.tensor_copy(xh[h][:, cs], xf[h][:, cs])

            theta_sb = acts.tile([O, N], dt.float16, tag="theta")
            phi_sb = acts.tile([O, N], dt.float16, tag="phi")
            gt_ones = acts.tile([128, 97 * NMC], dt.bfloat16, tag="gt")
            nc.vector.memset(gt_ones[:], 1.0)

            expS = expp.tile([128, NMC * QS], dt.bfloat16, tag="expS")
            cneg45 = consts.tile([128, 1], dt.float32, tag="cneg45")
            nc.vector.memset(cneg45[:], -45.0)

            # ---------------- P1: projections ----------------
            with tc.tile_pool(name="ps_proj", bufs=3, space="PSUM") as ps_proj, \
                 tc.tile_pool(name="ps_gtr", bufs=2, space="PSUM") as ps_gtr, \
                 tc.tile_pool(name="gsb", bufs=1) as gpool:
                g_sb = gpool.tile([O, N], dt.bfloat16, tag="gsb")
                for p, dst in (("theta", theta_sb), ("phi", phi_sb), ("g", g_sb)):
                    for j in range(NQ):
                        cs = slice(j * QS, (j + 1) * QS)
                        ps = ps_proj.tile([O, QS], dt.float32, tag="proj")
                        nc.tensor.matmul(ps[:], wt[p][:, 0:O], xh[0][:, cs],
                                         start=True, stop=False)
                        nc.tensor.matmul(ps[:], wt[p][:, O:2 * O], xh[1][:, cs],
                                         start=False, stop=True)
                        nc.vector.tensor_scalar_add(dst[:, cs], ps[:], bias[p][:])
                # transpose g -> gt_ones chunks [128m, 96] (col 96 stays 1.0)
                for mc in range(NMC):
                    tr = ps_gtr.tile([128, 96], dt.bfloat16, tag="gtr")
                    nc.tensor.transpose(tr[:], g_sb[:, 128 * mc:128 * mc + 128],
                                        idnbf[0:96, 0:96])
                    nc.vector.tensor_copy(
                        gt_ones[:, 97 * mc:97 * mc + 96], tr[:])

            # biased residual: xf += b_W (after xh conversion reads)
            for h in (0, 1):
                nc.vector.tensor_scalar_add(xf[h][:], xf[h][:], bW[h][:])

            # ---------------- P2: attention ----------------
            with tc.tile_pool(name="ps_qk", bufs=2, space="PSUM") as ps_qk, \
                 tc.tile_pool(name="ps_pv", bufs=1, space="PSUM") as ps_pv, \
                 tc.tile_pool(name="ps_ytr", bufs=1, space="PSUM") as ps_ytr, \
                 tc.tile_pool(name="ps_out", bufs=1, space="PSUM") as ps_out:
                for qs in range(NQ):
                    qcols = slice(qs * QS, (qs + 1) * QS)
                    ypsum = [ps_pv.tile([128, 97], dt.float32, tag=f"pv{st}", name=f"pv_{qs}_{st}")
                             for st in range(4)]
                    for mc in range(NMC):
                        ps = ps_qk.tile([128, QS], dt.float32, tag="qk")
                        nc.tensor.matmul(
                            ps[:], phi_sb[:, 128 * mc:128 * mc + 128],
                            theta_sb[:, qcols], start=True, stop=True)
                        nc.scalar.activation(
                            expS[:, 512 * mc:512 * mc + 512], ps[:],
                            AF.Exp, bias=cneg45[:])
                        for st in range(4):
                            nc.tensor.matmul(
                                ypsum[st][:],
                                expS[:, 512 * mc + 128 * st:512 * mc + 128 * st + 128],
                                gt_ones[:, 97 * mc:97 * mc + 97],
                                start=(mc == 0), stop=(mc == NMC - 1))
                    yt_sb = ytp.tile([O, QS], dt.float16, tag="yt")
                    for st in range(4):
                        linv = ypool.tile([128, 1], dt.float32, tag="linv")
                        nc.vector.reciprocal(linv[:], ypsum[st][:, 96:97])
                        yn = ypool.tile([128, 96], dt.float16, tag="yn")
                        nc.vector.tensor_scalar_mul(yn[:], ypsum[st][:, 0:96],
                                                    linv[:])
                        ytr = ps_ytr.tile([96, 128], dt.float16, tag="ytr")
                        nc.tensor.transpose(ytr[:], yn[:], idn16[:])
                        nc.vector.tensor_copy(
                            yt_sb[:, 128 * st:128 * st + 128], ytr[:])
                    for h in (0, 1):
                        pso = ps_out.tile([96, QS], dt.float32, tag="pout")
                        nc.tensor.matmul(pso[:], wWT[:, 96 * h:96 * h + 96],
                                         yt_sb[:], start=True, stop=True)
                        ob = outp.tile([96, QS], dt.float32, tag="ob")
                        nc.vector.tensor_add(ob[:], pso[:], xf[h][:, qcols])
                        nc.sync.dma_start(out_d[96 * h:96 * h + 96, qcols], ob[:])

    nc.compile()
    return nc


def _get_nc():
    if "nc" not in _CACHE:
        _CACHE["nc"] = _build()
    return _CACHE["nc"]


LAST_RESULTS = None


def kernel(x, g_w, g_b, theta_w, theta_b, phi_w, phi_b, W_w, W_b):
    global LAST_RESULTS
    from concourse.bass_utils import run_bass_kernel_spmd

    nc = _get_nc()
    f16 = ml_dtypes.float16 if hasattr(ml_dtypes, "float16") else np.float16

    x = np.asarray(x, dtype=np.float32)
    common = {
        "wt_theta": np.ascontiguousarray(np.asarray(theta_w).T).astype(np.float16),
        "wt_phi": np.ascontiguousarray(np.asarray(phi_w).T).astype(np.float16),
        "wt_g": np.ascontiguousarray(np.asarray(g_w).T).astype(np.float16),
        "w_WT": np.ascontiguousarray(np.asarray(W_w).T).astype(np.float16),
        "b_theta": np.asarray(theta_b, dtype=np.float32).reshape(O, 1),
        "b_phi": np.asarray(phi_b, dtype=np.float32).reshape(O, 1),
        "b_g": np.asarray(g_b, dtype=np.float32).reshape(O, 1),
        "b_W": np.asarray(W_b, dtype=np.float32).reshape(C, 1),
    }
    in_maps = [
        {"x": np.ascontiguousarray(x[b].reshape(C, N)), **common}
        for b in range(B)
    ]
    res = run_bass_kernel_spmd(nc, in_maps, list(range(N_CORES)))
    LAST_RESULTS = res
    out = np.stack([res.results[b]["out"].reshape(C, HH, WW) for b in range(B)])
    return out.astype(np.float32)
